# revision 5
# baseline (speedup 1.0000x reference)
"""Causal multi-head attention TRN2 kernel (8 NeuronCores).

Problem: B=4, S=2048, D=1024, H=16 heads, head_dim=64 (fp32 reference).

Sharding: data-parallel over batch (4) x tensor-parallel over head-groups (2).
Core c handles batch c//2 with heads (c%2)*8 .. (c%2)*8+8 and produces a
partial [S, D] output (its head-group's contribution to the O-projection,
without b_o, in bf16). Host sums the two partials per batch and adds b_o.

Single interleaved schedule (no phase barriers): Q/K/V projection granules,
attention score/exp/AV tiles, softmax normalization and O-projection tiles
are emitted in one dependency-driven stream so TensorE stays busy while
ScalarE (exp) and VectorE (evacuations) run underneath.

Per-core layout (everything pre-transposed on host; no on-chip transposes):
  xT        : SBUF [128, SB*KT*512] bf16, block-major slices of x^T
  qT/kT[j]  : [128, S] = (Wq pair-cols)^T @ x^T + bias; rows 0-63 head 2j,
              rows 64-127 head 2j+1
  v_all[t]  : [128, 8*65] bf16, v for all heads + ones column
  scores    : PSUM [128, 1024] = [K^T q | both heads] (row-tiled, concurrent)
  at        : SBUF bf16 = exp(scores/8); causal mask on diagonal tiles (DVE)
  wv        : PSUM [65, 1024] += [v|1]^T @ at (A|B halves; row 64 = sums)
  norm      : r = recip(sums) (DVE approx), DMA-broadcast to 64 partitions,
              two DVE mults -> wvT[j] (head B via partition-shift DMA)
  out       : PSUM [128,1024] += wvT^T @ Wo, evac bf16, DMA to DRAM
"""

import math

import numpy as np

B, S, D, H = 4, 2048, 1024, 16
HD = D // H        # 64
NCORES = 8
HPC = H // 2       # heads per core: 8
NPAIR = HPC // 2   # head pairs per core: 4
KT = D // 128      # contraction tiles: 8
ST = S // 128      # seq tiles of 128: 16
SB = S // 512      # seq blocks of 512: 4

_BUILT = {}
LAST_RESULTS = None  # BassKernelResults of the most recent run (for test.py)


def _build_nc():
    import concourse.bass as bass
    import concourse.mybir as mybir
    from concourse import tile

    f32 = mybir.dt.float32
    bf16 = mybir.dt.bfloat16
    AF = mybir.ActivationFunctionType
    OP = mybir.AluOpType

    nc = bass.Bass("TRN2", target_bir_lowering=False, debug=False,
                   num_devices=NCORES)

    # DRAM inputs, host-prearranged to match SBUF tile layouts exactly.
    xT_d = nc.dram_tensor("xT", [128, SB * KT * 512], bf16,
                          kind="ExternalInput").ap()
    wq_d = nc.dram_tensor("wq", [128, NPAIR * KT * 128], bf16,
                          kind="ExternalInput").ap()
    wk_d = nc.dram_tensor("wk", [128, NPAIR * KT * 128], bf16,
                          kind="ExternalInput").ap()
    wv_d = nc.dram_tensor("wv", [128, KT * 512], bf16,
                          kind="ExternalInput").ap()
    wo_d = nc.dram_tensor("wo", [128, NPAIR * D], bf16,
                          kind="ExternalInput").ap()
    bq_d = nc.dram_tensor("bq", [128, NPAIR], f32, kind="ExternalInput").ap()
    bk_d = nc.dram_tensor("bk", [128, NPAIR], f32, kind="ExternalInput").ap()
    bv_d = nc.dram_tensor("bv", [128, 512], f32, kind="ExternalInput").ap()
    mask_d = nc.dram_tensor("mask", [128, 512], bf16,
                            kind="ExternalInput").ap()
    out_d = nc.dram_tensor("out", [S, D], bf16, kind="ExternalOutput").ap()

    with tile.TileContext(nc) as tc:
        with tc.tile_pool(name="persist", bufs=1) as pp, \
             tc.tile_pool(name="psum", bufs=2, space="PSUM") as psp, \
             tc.tile_pool(name="attn", bufs=6) as atp, \
             tc.tile_pool(name="norm", bufs=2) as npl, \
             tc.tile_pool(name="scr", bufs=4, space="DRAM") as scrp, \
             tc.tile_pool(name="ost", bufs=3) as ostp:
            # ---- persistent SBUF tiles ----
            xt_all = pp.tile([128, SB * KT * 512], bf16, tag="xt")
            wq_all = pp.tile([128, NPAIR * KT * 128], bf16, tag="wq")
            wk_all = pp.tile([128, NPAIR * KT * 128], bf16, tag="wk")
            wv_all = pp.tile([128, KT * 512], bf16, tag="wv")
            wo_all = pp.tile([128, NPAIR * D], bf16, tag="wo")
            bq = pp.tile([128, NPAIR], f32, tag="bq")
            bk = pp.tile([128, NPAIR], f32, tag="bk")
            bv = pp.tile([128, 512], f32, tag="bv")
            mask = pp.tile([128, 512], bf16, tag="mask")
            qT = [pp.tile([128, S], bf16, tag=f"qT{j}", name=f"qT{j}")
                  for j in range(NPAIR)]
            kT = [pp.tile([128, S], bf16, tag=f"kT{j}", name=f"kT{j}")
                  for j in range(NPAIR)]
            v_all = [pp.tile([128, 8 * 65], bf16, tag=f"v{t}", name=f"v{t}")
                     for t in range(ST)]
            wvT = [pp.tile([128, S], bf16, tag=f"wvT{j}", name=f"wvT{j}")
                   for j in range(NPAIR)]

            # slice helpers
            def xt_s(c, k):  # [128, 512] x^T slice: rows 512c..512c+512 of x
                o = c * (KT * 512) + k * 512
                return xt_all[:, o:o + 512]

            def wq_s(j, k):
                o = j * (KT * 128) + k * 128
                return wq_all[:, o:o + 128]

            def wk_s(j, k):
                o = j * (KT * 128) + k * 128
                return wk_all[:, o:o + 128]

            wv_sl = [wv_all[:, 512 * k:512 * (k + 1)] for k in range(KT)]
            wo_sl = [wo_all[:, D * j:D * (j + 1)] for j in range(NPAIR)]

            # ---- input DMAs (critical-path-first ordering) ----
            jq = KT * 128
            cq = KT * 512
            nc.sync.dma_start(wq_all[:, 0:jq], wq_d[:, 0:jq])
            nc.sync.dma_start(wk_all[:, 0:jq], wk_d[:, 0:jq])
            nc.gpsimd.dma_start(xt_all[:, 0:cq], xT_d[:, 0:cq])
            nc.sync.dma_start(bq, bq_d[:, :])
            nc.sync.dma_start(bk, bk_d[:, :])
            nc.sync.dma_start(mask, mask_d[:, :])
            nc.sync.dma_start(bv, bv_d[:, :])
            nc.gpsimd.dma_start(wv_all, wv_d[:, :])
            nc.sync.dma_start(xt_all[:, cq:2 * cq], xT_d[:, cq:2 * cq])
            nc.gpsimd.dma_start(xt_all[:, 2 * cq:3 * cq],
                                xT_d[:, 2 * cq:3 * cq])
            nc.sync.dma_start(xt_all[:, 3 * cq:], xT_d[:, 3 * cq:])
            for j in range(1, NPAIR):
                nc.sync.dma_start(wq_all[:, j * jq:(j + 1) * jq],
                                  wq_d[:, j * jq:(j + 1) * jq])
                nc.sync.dma_start(wk_all[:, j * jq:(j + 1) * jq],
                                  wk_d[:, j * jq:(j + 1) * jq])
            nc.sync.dma_start(wo_all, wo_d[:, :])
            for t in range(ST):
                nc.gpsimd.memset(v_all[t][:, :], 1.0)

            # ---- granule emitters ----
            def emit_proj(j, c):
                cs = slice(512 * c, 512 * c + 512)
                pg = psp.tile([128, 1024], f32, tag="big", name=f"pj{j}_{c}")
                for k in range(KT):
                    nc.tensor.matmul(pg[:, 0:512], lhsT=wq_s(j, k),
                                     rhs=xt_s(c, k),
                                     start=(k == 0), stop=(k == KT - 1))
                for k in range(KT):
                    nc.tensor.matmul(pg[:, 512:1024], lhsT=wk_s(j, k),
                                     rhs=xt_s(c, k),
                                     start=(k == 0), stop=(k == KT - 1))
                nc.vector.tensor_scalar_add(qT[j][:, cs], pg[:, 0:512],
                                            bq[:, j:j + 1])
                nc.vector.tensor_scalar_add(kT[j][:, cs], pg[:, 512:1024],
                                            bk[:, j:j + 1])

            def emit_v2(t0):
                c, r0 = t0 // 4, t0 % 4
                pg = psp.tile([128, 1024], f32, tag="big", name=f"pv{t0}")
                for i in range(2):
                    hs = slice(512 * i, 512 * i + 512)
                    for k in range(KT):
                        nc.tensor.matmul(
                            pg[:, hs],
                            lhsT=xt_s(c, k)[:, 128 * (r0 + i):
                                            128 * (r0 + i) + 128],
                            rhs=wv_sl[k],
                            start=(k == 0), stop=(k == KT - 1))
                for i in range(2):
                    hs = slice(512 * i, 512 * i + 512)
                    nc.vector.tensor_tensor(
                        v_all[t0 + i].rearrange(
                            "p (h e) -> p h e", e=65)[:, :, 0:64],
                        pg[:, hs].rearrange("p (h e) -> p h e", e=64),
                        bv.rearrange("p (h e) -> p h e", e=64),
                        op=OP.add)

            def emit_O(s):
                ss = slice(128 * s, 128 * s + 128)
                pg = psp.tile([128, 1024], f32, tag="big", name=f"po{s}")
                for n in range(2):
                    ns = slice(512 * n, 512 * n + 512)
                    for j in range(NPAIR):
                        nc.tensor.matmul(pg[:, ns], lhsT=wvT[j][:, ss],
                                         rhs=wo_sl[j][:, ns],
                                         start=(j == 0),
                                         stop=(j == NPAIR - 1))
                ost = ostp.tile([128, 1024], bf16, tag="ost", name=f"ost{s}")
                nc.vector.tensor_copy(ost[:, :], pg[:, :])
                if s % 2 == 0:
                    nc.gpsimd.dma_start(out_d[ss, :], ost[:, :])
                else:
                    nc.sync.dma_start(out_d[ss, :], ost[:, :])

            pend = []  # deferred norm stage-B closures

            def emit_norm_b():
                while pend:
                    j, b, wv_ps, rb = pend.pop(0)
                    bs = slice(512 * b, 512 * b + 512)
                    nc.vector.tensor_tensor(
                        wvT[j][0:64, bs], wv_ps[0:64, 0:512],
                        rb[0:64, 0:512], op=OP.mult)
                    wvtmp = npl.tile([64, 512], bf16, tag="wvtmp",
                                     name=f"wvt{j}_{b}")
                    nc.vector.tensor_tensor(
                        wvtmp[:, :], wv_ps[0:64, 512:1024],
                        rb[0:64, 512:1024], op=OP.mult)
                    nc.gpsimd.dma_start(wvT[j][64:128, bs], wvtmp[:, :])

            def emit_attn(j, b, fillers):
                emit_norm_b()
                nt = 4 * b + 4
                # spread fillers across the t-loop
                fill_at = {}
                if fillers:
                    step = max(1, nt // len(fillers))
                    for i, f in enumerate(fillers):
                        fill_at.setdefault(min(nt - 1, 1 + i * step), []
                                           ).append(f)
                wv_ps = psp.tile([65, 1024], f32, tag="wv",
                                 name=f"wv{j}_{b}")
                for t in range(nt):
                    off = max(0, 128 * t - 512 * b)
                    w = 512 - off
                    ts_ = slice(128 * t, 128 * t + 128)
                    qs = slice(512 * b + off, 512 * b + 512)
                    pss = psp.tile([128, 1024], f32, tag="big",
                                   name=f"ps{j}_{b}_{t}")
                    nc.tensor.matmul(pss[:, off:512], lhsT=kT[j][0:64, ts_],
                                     rhs=qT[j][0:64, qs],
                                     start=True, stop=True,
                                     tile_position=(0, 0))
                    nc.tensor.matmul(pss[:, 512 + off:1024],
                                     lhsT=kT[j][64:128, ts_],
                                     rhs=qT[j][64:128, qs],
                                     start=True, stop=True,
                                     tile_position=(64, 0))
                    at = atp.tile([128, 1024], bf16, tag="at",
                                  name=f"at{j}_{b}_{t}")
                    if off:
                        nc.scalar.activation(
                            at.rearrange("p (h w) -> p h w",
                                         h=2)[:, :, off:512],
                            pss.rearrange("p (h w) -> p h w",
                                          h=2)[:, :, off:512],
                            AF.Exp, scale=0.125)
                    else:
                        nc.scalar.activation(at[:, :], pss[:, :], AF.Exp,
                                             scale=0.125)
                    if t >= 4 * b:
                        atw = at.rearrange("p (h w) -> p h w",
                                           h=2)[:, :, off:512]
                        msl = mask[:, None, 0:w].broadcast_to((128, 2, w))
                        nc.vector.tensor_tensor(atw, atw, msl, op=OP.mult)
                    for f in fill_at.get(t, ()):
                        f()
                    nc.tensor.matmul(wv_ps[:, off:512],
                                     lhsT=v_all[t][:, 130 * j:130 * j + 65],
                                     rhs=at[:, off:512],
                                     start=(t == 0), stop=(t == nt - 1))
                    nc.tensor.matmul(
                        wv_ps[:, 512 + off:1024],
                        lhsT=v_all[t][:, 130 * j + 65:130 * j + 130],
                        rhs=at[:, 512 + off:1024],
                        start=(t == 0), stop=(t == nt - 1))
                # norm stage A: reciprocal of sums row, broadcast to 64 rows
                r = npl.tile([1, 1024], f32, tag="r", name=f"r{j}_{b}",
                             bufs=3)
                nc.vector.reciprocal(r[:, :], wv_ps[64:65, :])
                scr = scrp.tile([1, 1024], f32, tag="scr", name=f"sc{j}_{b}")
                nc.sync.dma_start(scr[:, :], r[0:1, :])
                rb = npl.tile([64, 1024], f32, tag="rb", name=f"rb{j}_{b}")
                nc.sync.dma_start(rb[:, :],
                                  scr[0:1, :].broadcast_to((64, 1024)))
                pend.append((j, b, wv_ps, rb))

            # ---- the schedule ----
            emit_proj(0, 0)
            emit_v2(0)
            emit_v2(2)
            F = lambda f, *a: (lambda: f(*a))
            emit_attn(0, 0, [F(emit_proj, 0, 1), F(emit_v2, 4),
                             F(emit_v2, 6)])
            emit_attn(0, 1, [F(emit_proj, 0, 2), F(emit_v2, 8),
                             F(emit_v2, 10)])
            emit_attn(0, 2, [F(emit_proj, 0, 3), F(emit_v2, 12),
                             F(emit_v2, 14)])
            emit_attn(0, 3, [F(emit_proj, 1, 0), F(emit_proj, 1, 1),
                             F(emit_proj, 1, 2)])
            emit_attn(1, 0, [F(emit_proj, 1, 3)])
            emit_attn(1, 1, [F(emit_proj, 2, 0)])
            emit_attn(1, 2, [F(emit_proj, 2, 1)])
            emit_attn(1, 3, [F(emit_proj, 2, 2), F(emit_proj, 2, 3)])
            emit_attn(2, 0, [F(emit_proj, 3, 0)])
            emit_attn(2, 1, [F(emit_proj, 3, 1)])
            emit_attn(2, 2, [F(emit_proj, 3, 2)])
            emit_attn(2, 3, [F(emit_proj, 3, 3)])
            emit_attn(3, 0, [])
            emit_attn(3, 1, [F(emit_O, 0), F(emit_O, 1), F(emit_O, 2),
                             F(emit_O, 3)])
            emit_attn(3, 2, [F(emit_O, 4), F(emit_O, 5), F(emit_O, 6),
                             F(emit_O, 7)])
            emit_attn(3, 3, [F(emit_O, 8), F(emit_O, 9), F(emit_O, 10),
                             F(emit_O, 11)])
            emit_norm_b()
            for s in range(12, 16):
                emit_O(s)
    _split_excess_waits(nc, limit=1)
    return nc


def _split_excess_waits(nc, limit=1):
    """This container's walrus encodes at most one sem wait per instruction;
    move excess waits onto standalone EventSemaphore ops just before each
    over-limit instruction (same engine stream, so semantics preserved)."""
    import concourse.mybir as mybir
    n = 0
    for fn in nc.m.functions:
        for bb in fn.blocks:
            new_insts = []
            for inst in bb.instructions:
                si = inst.sync_info
                if si is not None and si.on_wait and len(si.on_wait) > limit:
                    waits = list(si.on_wait)
                    for i, w in enumerate(waits[limit:]):
                        wi = mybir.InstEventSemaphore(
                            name=f"{inst.name}-wsplit{i}", ins=[], outs=[],
                            sync_info=mybir.SyncInfo(on_wait=[w], on_update=[]))
                        wi.engine = inst.engine
                        nc.register_instruction(wi)
                        new_insts.append(wi)
                        n += 1
                    si.on_wait = waits[:limit]
                new_insts.append(inst)
            bb.instructions = new_insts
    return n


def _get_nc():
    if "nc" not in _BUILT:
        _BUILT["nc"] = _build_nc()
    return _BUILT["nc"]


def _prep_core_inputs(x_b, W_q, b_q, W_k, b_k, W_v, b_v, W_o, g):
    """Inputs for one core: batch slice x_b [S, D], head group g (0/1)."""
    import ml_dtypes
    bf16 = ml_dtypes.bfloat16
    hs = slice(g * HPC, (g + 1) * HPC)

    # xT block-major: [p, (c, k, u)] = x_b[512c+u, 128k+p]
    xT = np.ascontiguousarray(
        x_b.T.reshape(KT, 128, SB, 512).transpose(1, 2, 0, 3).reshape(
            128, SB * KT * 512)).astype(bf16)

    def arrange_qk(w):  # [D, 512] -> [128, (j, k, 128)]
        return np.ascontiguousarray(
            w.reshape(KT, 128, NPAIR, 128).transpose(1, 2, 0, 3).reshape(
                128, NPAIR * KT * 128))

    wq = arrange_qk(
        W_q[hs].transpose(1, 0, 2).reshape(D, 512)).astype(bf16)
    wk = arrange_qk(
        W_k[hs].transpose(1, 0, 2).reshape(D, 512)).astype(bf16)
    wv = np.ascontiguousarray(
        W_v[hs].transpose(1, 0, 2).reshape(D, 512)
        .reshape(KT, 128, 512).transpose(1, 0, 2).reshape(128, KT * 512)
    ).astype(bf16)
    wo_t = np.ascontiguousarray(W_o[:, g * 512:(g + 1) * 512].T)  # [512, D]
    wo = np.ascontiguousarray(
        wo_t.reshape(NPAIR, 128, D).transpose(1, 0, 2).reshape(128, NPAIR * D)
    ).astype(bf16)
    bq = np.ascontiguousarray(
        b_q[hs].reshape(NPAIR, 128).T).astype(np.float32)          # [128, 4]
    bk = np.ascontiguousarray(
        b_k[hs].reshape(NPAIR, 128).T).astype(np.float32)
    bv = np.ascontiguousarray(np.broadcast_to(
        b_v[hs].reshape(1, 512), (128, 512))).astype(np.float32)   # [128, 512]

    p = np.arange(128)[:, None]
    u = np.arange(512)[None, :]
    mask = (u >= p).astype(bf16)                                   # [128, 512]

    return {"xT": xT, "wq": wq, "wk": wk, "wv": wv, "wo": wo,
            "bq": bq, "bk": bk, "bv": bv, "mask": mask}


def _install_axon_ntff_hook():
    """Register the axon NTFF profiling hook if the environment allows.

    The agent image lacks ``antenv.axon_hooks``; synthesize it and wire the
    ctypes-based profiler from trn_agent_boot so BASS_TRACE=1 yields NTFFs.
    Degrades silently — without it run_bass_kernel_spmd(trace=False) works.
    """
    import sys
    import types
    try:
        import antenv
        if "antenv.axon_hooks" not in sys.modules:
            mod = types.ModuleType("antenv.axon_hooks")
            holder = [None]
            mod.set_axon_ntff_profile_hook = lambda h: holder.__setitem__(0, h)
            mod.get_axon_ntff_profile_hook = lambda: holder[0]
            sys.modules["antenv.axon_hooks"] = mod
            antenv.axon_hooks = mod
        mod = sys.modules["antenv.axon_hooks"]
        if mod.get_axon_ntff_profile_hook() is None:
            from trn_agent_boot.trn_boot import _ntff_profile_via_ctypes
            hook = _ntff_profile_via_ctypes("/opt/axon/libaxon_pjrt.so")
            mod.set_axon_ntff_profile_hook(hook)
        import concourse.bass_utils as bu
        bu.upload_artifacts = lambda d: d  # no S3 in this container
    except Exception:
        pass


def kernel(inputs, W_q, b_q, W_k, b_k, W_v, b_v, W_o, b_o):
    global LAST_RESULTS
    from concourse.bass_utils import run_bass_kernel_spmd
    _install_axon_ntff_hook()

    inputs = np.asarray(inputs, dtype=np.float32)
    args = [np.asarray(a, dtype=np.float32)
            for a in (W_q, b_q, W_k, b_k, W_v, b_v, W_o, b_o)]
    W_q, b_q, W_k, b_k, W_v, b_v, W_o, b_o = args

    nc = _get_nc()
    in_maps = []
    for c in range(NCORES):
        bi, g = c // 2, c % 2
        in_maps.append(_prep_core_inputs(
            inputs[bi], W_q, b_q, W_k, b_k, W_v, b_v, W_o, g))

    res = run_bass_kernel_spmd(nc, in_maps, list(range(NCORES)))
    LAST_RESULTS = res

    out = np.empty((B, S, D), dtype=np.float32)
    for bi in range(B):
        out[bi] = (res.results[2 * bi]["out"].astype(np.float32)
                   + res.results[2 * bi + 1]["out"].astype(np.float32)
                   + b_o[None, :])
    return out


# revision 6
# speedup vs baseline: 1.1790x; 1.1790x over previous
"""Causal multi-head attention TRN2 kernel (8 NeuronCores).

Problem: B=4, S=2048, D=1024, H=16 heads, head_dim=64 (fp32 reference).

Sharding: data-parallel over batch (4) x tensor-parallel over head-groups (2).
Core c handles batch c//2 with heads (c%2)*8 .. (c%2)*8+8 and produces a
partial [S, D] output (its head-group's contribution to the O-projection,
without b_o, in bf16). Host sums the two partials per batch and adds b_o.

Single interleaved schedule (no phase barriers): Q/K/V projection granules,
attention score/exp/AV tiles, softmax normalization and O-projection tiles
are emitted in one dependency-driven stream so TensorE stays busy while
ScalarE (exp) and VectorE (evacuations) run underneath.

Per-core layout (everything pre-transposed on host; no on-chip transposes):
  xT        : SBUF [128, SB*KT*512] bf16, block-major slices of x^T
  qT/kT[j]  : [128, S] = (Wq pair-cols)^T @ x^T + bias; rows 0-63 head 2j,
              rows 64-127 head 2j+1
  v_all[t]  : [128, 8*65] bf16, v for all heads + ones column
  scores    : PSUM [128, 1024] = [K^T q | both heads] (row-tiled, concurrent)
  at        : SBUF bf16 = exp(scores/8); causal mask on diagonal tiles (DVE)
  wv        : PSUM [65, 1024] += [v|1]^T @ at (A|B halves; row 64 = sums)
  norm      : r = recip(sums) (DVE approx), DMA-broadcast to 64 partitions,
              two DVE mults -> wvT[j] (head B via partition-shift DMA)
  out       : PSUM [128,1024] += wvT^T @ Wo, evac bf16, DMA to DRAM
"""

import math

import numpy as np

B, S, D, H = 4, 2048, 1024, 16
HD = D // H        # 64
NCORES = 8
HPC = H // 2       # heads per core: 8
NPAIR = HPC // 2   # head pairs per core: 4
KT = D // 128      # contraction tiles: 8
ST = S // 128      # seq tiles of 128: 16
SB = S // 512      # seq blocks of 512: 4

_BUILT = {}
LAST_RESULTS = None  # BassKernelResults of the most recent run (for test.py)


def _build_nc():
    import concourse.bass as bass
    import concourse.mybir as mybir
    from concourse import tile

    f32 = mybir.dt.float32
    bf16 = mybir.dt.bfloat16
    AF = mybir.ActivationFunctionType
    OP = mybir.AluOpType

    nc = bass.Bass("TRN2", target_bir_lowering=False, debug=False,
                   num_devices=NCORES)

    # DRAM inputs, host-prearranged to match SBUF tile layouts exactly.
    xT_d = nc.dram_tensor("xT", [128, SB * KT * 512], bf16,
                          kind="ExternalInput").ap()
    wq_d = nc.dram_tensor("wq", [128, NPAIR * KT * 128], bf16,
                          kind="ExternalInput").ap()
    wk_d = nc.dram_tensor("wk", [128, NPAIR * KT * 128], bf16,
                          kind="ExternalInput").ap()
    wv_d = nc.dram_tensor("wv", [128, KT * 512], bf16,
                          kind="ExternalInput").ap()
    wo_d = nc.dram_tensor("wo", [128, NPAIR * D], bf16,
                          kind="ExternalInput").ap()
    bq_d = nc.dram_tensor("bq", [128, NPAIR], f32, kind="ExternalInput").ap()
    bk_d = nc.dram_tensor("bk", [128, NPAIR], f32, kind="ExternalInput").ap()
    bv_d = nc.dram_tensor("bv", [128, 512], f32, kind="ExternalInput").ap()
    mask_d = nc.dram_tensor("mask", [128, 512], bf16,
                            kind="ExternalInput").ap()
    out_d = nc.dram_tensor("out", [S, D], bf16, kind="ExternalOutput").ap()

    with tile.TileContext(nc) as tc:
        with tc.tile_pool(name="persist", bufs=1) as pp, \
             tc.tile_pool(name="psum", bufs=2, space="PSUM") as psp, \
             tc.tile_pool(name="attn", bufs=6) as atp, \
             tc.tile_pool(name="norm", bufs=2) as npl, \
             tc.tile_pool(name="scr", bufs=4, space="DRAM") as scrp, \
             tc.tile_pool(name="ost", bufs=3) as ostp:
            # ---- persistent SBUF tiles ----
            xt_all = pp.tile([128, SB * KT * 512], bf16, tag="xt")
            wq_all = pp.tile([128, NPAIR * KT * 128], bf16, tag="wq")
            wk_all = pp.tile([128, NPAIR * KT * 128], bf16, tag="wk")
            wv_all = pp.tile([128, KT * 512], bf16, tag="wv")
            wo_all = pp.tile([128, NPAIR * D], bf16, tag="wo")
            bq = pp.tile([128, NPAIR], f32, tag="bq")
            bk = pp.tile([128, NPAIR], f32, tag="bk")
            bv = pp.tile([128, 512], f32, tag="bv")
            mask = pp.tile([128, 512], bf16, tag="mask")
            qT = [pp.tile([128, S], bf16, tag=f"qT{j}", name=f"qT{j}")
                  for j in range(NPAIR)]
            kT = [pp.tile([128, S], bf16, tag=f"kT{j}", name=f"kT{j}")
                  for j in range(NPAIR)]
            v_all = [pp.tile([128, 8 * 65], bf16, tag=f"v{t}", name=f"v{t}")
                     for t in range(ST)]
            wvT = [pp.tile([128, S], bf16, tag=f"wvT{j}", name=f"wvT{j}")
                   for j in range(NPAIR)]

            # slice helpers
            def xt_s(c, k):  # [128, 512] x^T slice: rows 512c..512c+512 of x
                o = c * (KT * 512) + k * 512
                return xt_all[:, o:o + 512]

            def wq_s(j, k):
                o = j * (KT * 128) + k * 128
                return wq_all[:, o:o + 128]

            def wk_s(j, k):
                o = j * (KT * 128) + k * 128
                return wk_all[:, o:o + 128]

            wv_sl = [wv_all[:, 512 * k:512 * (k + 1)] for k in range(KT)]
            wo_sl = [wo_all[:, D * j:D * (j + 1)] for j in range(NPAIR)]

            # ---- input DMAs (critical-path-first ordering) ----
            jq = KT * 128
            cq = KT * 512
            nc.sync.dma_start(wq_all[:, 0:jq], wq_d[:, 0:jq])
            nc.sync.dma_start(wk_all[:, 0:jq], wk_d[:, 0:jq])
            nc.gpsimd.dma_start(xt_all[:, 0:cq], xT_d[:, 0:cq])
            nc.sync.dma_start(bq, bq_d[:, :])
            nc.sync.dma_start(bk, bk_d[:, :])
            nc.sync.dma_start(mask, mask_d[:, :])
            nc.sync.dma_start(bv, bv_d[:, :])
            nc.gpsimd.dma_start(wv_all, wv_d[:, :])
            nc.sync.dma_start(xt_all[:, cq:2 * cq], xT_d[:, cq:2 * cq])
            nc.gpsimd.dma_start(xt_all[:, 2 * cq:3 * cq],
                                xT_d[:, 2 * cq:3 * cq])
            nc.sync.dma_start(xt_all[:, 3 * cq:], xT_d[:, 3 * cq:])
            for j in range(1, NPAIR):
                nc.sync.dma_start(wq_all[:, j * jq:(j + 1) * jq],
                                  wq_d[:, j * jq:(j + 1) * jq])
                nc.sync.dma_start(wk_all[:, j * jq:(j + 1) * jq],
                                  wk_d[:, j * jq:(j + 1) * jq])
            nc.sync.dma_start(wo_all, wo_d[:, :])
            for t in range(ST):
                nc.gpsimd.memset(v_all[t][:, :], 1.0)

            # ---- granule emitters ----
            def emit_proj(j, c):
                cs = slice(512 * c, 512 * c + 512)
                pg = psp.tile([128, 1024], f32, tag="big", name=f"pj{j}_{c}")
                for k in range(KT):
                    nc.tensor.matmul(pg[:, 0:512], lhsT=wq_s(j, k),
                                     rhs=xt_s(c, k),
                                     start=(k == 0), stop=(k == KT - 1))
                for k in range(KT):
                    nc.tensor.matmul(pg[:, 512:1024], lhsT=wk_s(j, k),
                                     rhs=xt_s(c, k),
                                     start=(k == 0), stop=(k == KT - 1))
                nc.vector.tensor_scalar_add(qT[j][:, cs], pg[:, 0:512],
                                            bq[:, j:j + 1])
                nc.vector.tensor_scalar_add(kT[j][:, cs], pg[:, 512:1024],
                                            bk[:, j:j + 1])

            def emit_v2(t0):
                c, r0 = t0 // 4, t0 % 4
                pg = psp.tile([128, 1024], f32, tag="big", name=f"pv{t0}")
                for i in range(2):
                    hs = slice(512 * i, 512 * i + 512)
                    for k in range(KT):
                        nc.tensor.matmul(
                            pg[:, hs],
                            lhsT=xt_s(c, k)[:, 128 * (r0 + i):
                                            128 * (r0 + i) + 128],
                            rhs=wv_sl[k],
                            start=(k == 0), stop=(k == KT - 1))
                for i in range(2):
                    hs = slice(512 * i, 512 * i + 512)
                    nc.vector.tensor_tensor(
                        v_all[t0 + i].rearrange(
                            "p (h e) -> p h e", e=65)[:, :, 0:64],
                        pg[:, hs].rearrange("p (h e) -> p h e", e=64),
                        bv.rearrange("p (h e) -> p h e", e=64),
                        op=OP.add)

            def emit_O(s):
                ss = slice(128 * s, 128 * s + 128)
                pg = psp.tile([128, 1024], f32, tag="big", name=f"po{s}")
                for n in range(2):
                    ns = slice(512 * n, 512 * n + 512)
                    for j in range(NPAIR):
                        nc.tensor.matmul(pg[:, ns], lhsT=wvT[j][:, ss],
                                         rhs=wo_sl[j][:, ns],
                                         start=(j == 0),
                                         stop=(j == NPAIR - 1))
                ost = ostp.tile([128, 1024], bf16, tag="ost", name=f"ost{s}")
                nc.vector.tensor_copy(ost[:, :], pg[:, :])
                if s % 2 == 0:
                    nc.gpsimd.dma_start(out_d[ss, :], ost[:, :])
                else:
                    nc.sync.dma_start(out_d[ss, :], ost[:, :])

            pend = []  # deferred norm stage-B closures

            def emit_norm_b():
                while pend:
                    j, b, wv_ps, rb = pend.pop(0)
                    bs = slice(512 * b, 512 * b + 512)
                    nc.vector.tensor_tensor(
                        wvT[j][0:64, bs], wv_ps[0:64, 0:512],
                        rb[0:64, 0:512], op=OP.mult)
                    wvtmp = npl.tile([64, 512], bf16, tag="wvtmp",
                                     name=f"wvt{j}_{b}")
                    nc.vector.tensor_tensor(
                        wvtmp[:, :], wv_ps[0:64, 512:1024],
                        rb[0:64, 512:1024], op=OP.mult)
                    nc.gpsimd.dma_start(wvT[j][64:128, bs], wvtmp[:, :])

            def emit_attn(j, b, fillers):
                emit_norm_b()
                nt = 4 * b + 4
                # spread fillers across the t-loop
                fill_at = {}
                if fillers:
                    step = max(1, nt // len(fillers))
                    for i, f in enumerate(fillers):
                        fill_at.setdefault(min(nt - 1, 1 + i * step), []
                                           ).append(f)
                wv_ps = psp.tile([65, 1024], f32, tag="wv",
                                 name=f"wv{j}_{b}")
                for t in range(nt):
                    off = max(0, 128 * t - 512 * b)
                    w = 512 - off
                    ts_ = slice(128 * t, 128 * t + 128)
                    qs = slice(512 * b + off, 512 * b + 512)
                    pss = psp.tile([128, 1024], f32, tag="big",
                                   name=f"ps{j}_{b}_{t}")
                    nc.tensor.matmul(pss[:, off:512], lhsT=kT[j][0:64, ts_],
                                     rhs=qT[j][0:64, qs],
                                     start=True, stop=True,
                                     tile_position=(0, 0))
                    nc.tensor.matmul(pss[:, 512 + off:1024],
                                     lhsT=kT[j][64:128, ts_],
                                     rhs=qT[j][64:128, qs],
                                     start=True, stop=True,
                                     tile_position=(64, 0))
                    at = atp.tile([128, 1024], bf16, tag="at",
                                  name=f"at{j}_{b}_{t}")
                    if off:
                        nc.scalar.activation(
                            at.rearrange("p (h w) -> p h w",
                                         h=2)[:, :, off:512],
                            pss.rearrange("p (h w) -> p h w",
                                          h=2)[:, :, off:512],
                            AF.Exp, scale=0.125)
                    else:
                        nc.scalar.activation(at[:, :], pss[:, :], AF.Exp,
                                             scale=0.125)
                    if t >= 4 * b:
                        atw = at.rearrange("p (h w) -> p h w",
                                           h=2)[:, :, off:512]
                        msl = mask[:, None, 0:w].broadcast_to((128, 2, w))
                        nc.vector.tensor_tensor(atw, atw, msl, op=OP.mult)
                    for f in fill_at.get(t, ()):
                        f()
                    nc.tensor.matmul(wv_ps[:, off:512],
                                     lhsT=v_all[t][:, 130 * j:130 * j + 65],
                                     rhs=at[:, off:512],
                                     start=(t == 0), stop=(t == nt - 1))
                    nc.tensor.matmul(
                        wv_ps[:, 512 + off:1024],
                        lhsT=v_all[t][:, 130 * j + 65:130 * j + 130],
                        rhs=at[:, 512 + off:1024],
                        start=(t == 0), stop=(t == nt - 1))
                # norm stage A: sums row -> 128 partitions, recip, DRAM
                # bounce, broadcast to 64 rows
                srow = npl.tile([1, 1024], f32, tag="srow",
                                name=f"sr{j}_{b}", bufs=3)
                nc.vector.tensor_copy(srow[:, :], wv_ps[64:65, :])
                sq = npl.tile([128, 8], f32, tag="sq", name=f"sq{j}_{b}",
                              bufs=3)
                nc.sync.dma_start(sq[:, :], srow[0:1, :])
                rq = npl.tile([128, 8], f32, tag="rq", name=f"rq{j}_{b}",
                              bufs=3)
                nc.vector.reciprocal(rq[:, :], sq[:, :])
                scr = scrp.tile([1, 1024], f32, tag="scr", name=f"sc{j}_{b}")
                nc.sync.dma_start(scr[:, :], rq[:, :])
                rb = npl.tile([64, 1024], f32, tag="rb", name=f"rb{j}_{b}")
                nc.sync.dma_start(rb[:, :],
                                  scr[0:1, :].broadcast_to((64, 1024)))
                pend.append((j, b, wv_ps, rb))

            # ---- the schedule ----
            emit_proj(0, 0)
            emit_v2(0)
            emit_v2(2)
            F = lambda f, *a: (lambda: f(*a))
            emit_attn(0, 0, [F(emit_proj, 0, 1), F(emit_v2, 4),
                             F(emit_v2, 6)])
            emit_attn(0, 1, [F(emit_proj, 0, 2), F(emit_v2, 8),
                             F(emit_v2, 10)])
            emit_attn(0, 2, [F(emit_proj, 0, 3), F(emit_v2, 12),
                             F(emit_v2, 14)])
            emit_attn(0, 3, [F(emit_proj, 1, 0), F(emit_proj, 1, 1),
                             F(emit_proj, 1, 2)])
            emit_attn(1, 0, [F(emit_proj, 1, 3)])
            emit_attn(1, 1, [F(emit_proj, 2, 0)])
            emit_attn(1, 2, [F(emit_proj, 2, 1)])
            emit_attn(1, 3, [F(emit_proj, 2, 2), F(emit_proj, 2, 3)])
            emit_attn(2, 0, [F(emit_proj, 3, 0)])
            emit_attn(2, 1, [F(emit_proj, 3, 1)])
            emit_attn(2, 2, [F(emit_proj, 3, 2)])
            emit_attn(2, 3, [F(emit_proj, 3, 3)])
            emit_attn(3, 0, [])
            emit_attn(3, 1, [F(emit_O, 0), F(emit_O, 1), F(emit_O, 2),
                             F(emit_O, 3)])
            emit_attn(3, 2, [F(emit_O, 4), F(emit_O, 5), F(emit_O, 6),
                             F(emit_O, 7)])
            emit_attn(3, 3, [F(emit_O, 8), F(emit_O, 9), F(emit_O, 10),
                             F(emit_O, 11)])
            emit_norm_b()
            for s in range(12, 16):
                emit_O(s)
    _split_excess_waits(nc, limit=1)
    return nc


def _split_excess_waits(nc, limit=1):
    """This container's walrus encodes at most one sem wait per instruction;
    move excess waits onto standalone EventSemaphore ops just before each
    over-limit instruction (same engine stream, so semantics preserved)."""
    import concourse.mybir as mybir
    n = 0
    for fn in nc.m.functions:
        for bb in fn.blocks:
            new_insts = []
            for inst in bb.instructions:
                si = inst.sync_info
                if si is not None and si.on_wait and len(si.on_wait) > limit:
                    waits = list(si.on_wait)
                    for i, w in enumerate(waits[limit:]):
                        wi = mybir.InstEventSemaphore(
                            name=f"{inst.name}-wsplit{i}", ins=[], outs=[],
                            sync_info=mybir.SyncInfo(on_wait=[w], on_update=[]))
                        wi.engine = inst.engine
                        nc.register_instruction(wi)
                        new_insts.append(wi)
                        n += 1
                    si.on_wait = waits[:limit]
                new_insts.append(inst)
            bb.instructions = new_insts
    return n


def _get_nc():
    if "nc" not in _BUILT:
        _BUILT["nc"] = _build_nc()
    return _BUILT["nc"]


def _prep_core_inputs(x_b, W_q, b_q, W_k, b_k, W_v, b_v, W_o, g):
    """Inputs for one core: batch slice x_b [S, D], head group g (0/1)."""
    import ml_dtypes
    bf16 = ml_dtypes.bfloat16
    hs = slice(g * HPC, (g + 1) * HPC)

    # xT block-major: [p, (c, k, u)] = x_b[512c+u, 128k+p]
    xT = np.ascontiguousarray(
        x_b.T.reshape(KT, 128, SB, 512).transpose(1, 2, 0, 3).reshape(
            128, SB * KT * 512)).astype(bf16)

    def arrange_qk(w):  # [D, 512] -> [128, (j, k, 128)]
        return np.ascontiguousarray(
            w.reshape(KT, 128, NPAIR, 128).transpose(1, 2, 0, 3).reshape(
                128, NPAIR * KT * 128))

    wq = arrange_qk(
        W_q[hs].transpose(1, 0, 2).reshape(D, 512)).astype(bf16)
    wk = arrange_qk(
        W_k[hs].transpose(1, 0, 2).reshape(D, 512)).astype(bf16)
    wv = np.ascontiguousarray(
        W_v[hs].transpose(1, 0, 2).reshape(D, 512)
        .reshape(KT, 128, 512).transpose(1, 0, 2).reshape(128, KT * 512)
    ).astype(bf16)
    wo_t = np.ascontiguousarray(W_o[:, g * 512:(g + 1) * 512].T)  # [512, D]
    wo = np.ascontiguousarray(
        wo_t.reshape(NPAIR, 128, D).transpose(1, 0, 2).reshape(128, NPAIR * D)
    ).astype(bf16)
    bq = np.ascontiguousarray(
        b_q[hs].reshape(NPAIR, 128).T).astype(np.float32)          # [128, 4]
    bk = np.ascontiguousarray(
        b_k[hs].reshape(NPAIR, 128).T).astype(np.float32)
    bv = np.ascontiguousarray(np.broadcast_to(
        b_v[hs].reshape(1, 512), (128, 512))).astype(np.float32)   # [128, 512]

    p = np.arange(128)[:, None]
    u = np.arange(512)[None, :]
    mask = (u >= p).astype(bf16)                                   # [128, 512]

    return {"xT": xT, "wq": wq, "wk": wk, "wv": wv, "wo": wo,
            "bq": bq, "bk": bk, "bv": bv, "mask": mask}


def _install_axon_ntff_hook():
    """Register the axon NTFF profiling hook if the environment allows.

    The agent image lacks ``antenv.axon_hooks``; synthesize it and wire the
    ctypes-based profiler from trn_agent_boot so BASS_TRACE=1 yields NTFFs.
    Degrades silently — without it run_bass_kernel_spmd(trace=False) works.
    """
    import sys
    import types
    try:
        import antenv
        if "antenv.axon_hooks" not in sys.modules:
            mod = types.ModuleType("antenv.axon_hooks")
            holder = [None]
            mod.set_axon_ntff_profile_hook = lambda h: holder.__setitem__(0, h)
            mod.get_axon_ntff_profile_hook = lambda: holder[0]
            sys.modules["antenv.axon_hooks"] = mod
            antenv.axon_hooks = mod
        mod = sys.modules["antenv.axon_hooks"]
        if mod.get_axon_ntff_profile_hook() is None:
            from trn_agent_boot.trn_boot import _ntff_profile_via_ctypes
            hook = _ntff_profile_via_ctypes("/opt/axon/libaxon_pjrt.so")
            mod.set_axon_ntff_profile_hook(hook)
        import concourse.bass_utils as bu
        bu.upload_artifacts = lambda d: d  # no S3 in this container
    except Exception:
        pass


def kernel(inputs, W_q, b_q, W_k, b_k, W_v, b_v, W_o, b_o):
    global LAST_RESULTS
    from concourse.bass_utils import run_bass_kernel_spmd
    _install_axon_ntff_hook()

    inputs = np.asarray(inputs, dtype=np.float32)
    args = [np.asarray(a, dtype=np.float32)
            for a in (W_q, b_q, W_k, b_k, W_v, b_v, W_o, b_o)]
    W_q, b_q, W_k, b_k, W_v, b_v, W_o, b_o = args

    nc = _get_nc()
    in_maps = []
    for c in range(NCORES):
        bi, g = c // 2, c % 2
        in_maps.append(_prep_core_inputs(
            inputs[bi], W_q, b_q, W_k, b_k, W_v, b_v, W_o, g))

    res = run_bass_kernel_spmd(nc, in_maps, list(range(NCORES)))
    LAST_RESULTS = res

    out = np.empty((B, S, D), dtype=np.float32)
    for bi in range(B):
        out[bi] = (res.results[2 * bi]["out"].astype(np.float32)
                   + res.results[2 * bi + 1]["out"].astype(np.float32)
                   + b_o[None, :])
    return out
